# revision 11
# baseline (speedup 1.0000x reference)
"""LIF (leaky integrate-and-fire) spiking recurrence on 8 Trainium2 cores.

Full input x: [T*bs, C, H, W] = [256, 128, 32, 32] f32 with T=8, bs=32.
Recurrence over T only, elementwise elsewhere:
    u_t = TAU * u_{t-1} * (1 - (u_{t-1} > VTH)) + x_t ;  o_t = (u_t > VTH)

Sharding: fully data-parallel over batch (bs=32 -> 4 per core), no
collectives. Each core's [4,128,32,32] per-timestep slab is a flat
[128 partitions, 4096] tile; the whole 16 MiB input stays SBUF-resident
and the state is computed in place over it.

Structure (per-core DMA floor ~50us at the ~426 GB/s cap):

1. Byte spikes: output is pure 0/1, so ACT computes s = sign(v - 2^t) as
   uint8 in one pass; host maps byte==1 -> 1.0f (exact under any
   float->u8 conversion semantics). Store traffic 16.78 -> 4.19 MB/core.

2. Scaled state: with v_t = 2^t * u_t (host pre-scales X_t = 2^t * x_t;
   powers of two, recurrence bitwise identical), the TAU decay vanishes:
       w_t = v_t * (v_t <= 2^t)     (mask: DVE scalar_tensor_tensor)
       v_{t+1} = w_t + X_{t+1}      (pure elementwise add)

3. DVE+GPSIMD split: GPSIMD (8 Q7 DSP cores, ~2.7ns/elem tensor_tensor)
   executes the update add for columns [DW:4096) in two chunks, DVE adds
   [0:DW) and runs all masks (GPSIMD-region chunks first so the
   cross-engine chain pipelines). The 62us DVE-only elementwise load
   drops to ~6.7us DVE + ~6.2us GPSIMD per step, overlapped.

Everything except the add is exact (power-of-two scales, 0/1 masks), so
the result is bitwise identical to the f32 reference.
"""

import numpy as np

import concourse.tile as tile
from concourse import bacc, mybir
from concourse.bass_utils import run_bass_kernel_spmd

T = 8
BS = 32
C = 128
HW = 32 * 32
NCORES = 8
BSH = BS // NCORES          # 4 batch elements per core
P = 128                     # SBUF partitions
FREE = BSH * C * HW // P    # 4096 f32 per partition per timestep
CHF = 2048                  # load-ramp chunk (half timestep)
DW = 1792                   # DVE-owned update columns
GW = (FREE - DW) // 2       # 1152: each of GPSIMD's two update chunks
VTH = 1.0
F32 = mybir.dt.float32
U8 = mybir.dt.uint8

_nc_cache = None


def _build():
    nc = bacc.Bacc("TRN2", target_bir_lowering=False, debug=False, num_devices=NCORES)
    x_d = nc.dram_tensor("x", [T, P, FREE], F32, kind="ExternalInput").ap()
    o_d = nc.dram_tensor("o", [T, P, FREE], U8, kind="ExternalOutput").ap()

    ADD = mybir.AluOpType.add
    ISLE = mybir.AluOpType.is_le
    MULT = mybir.AluOpType.mult
    SIGN = mybir.ActivationFunctionType.Sign

    # Column spans: D = DVE's update region, G0/G1 = GPSIMD's chunks.
    spans = [(0, DW), (DW, DW + GW), (DW + GW, FREE)]

    with tile.TileContext(nc) as tc:
        with (
            tc.tile_pool(name="xa", bufs=1) as xa,
            tc.tile_pool(name="pp", bufs=2) as pp,
            tc.tile_pool(name="op", bufs=2) as op,
            tc.tile_pool(name="cp", bufs=1) as cp,
        ):
            nbias = cp.tile([P, T], F32)
            for t in range(T):
                nc.vector.memset(nbias[:, t:t + 1], -float(2 ** t))

            xt = xa.tile([P, T * FREE], F32)
            xv = x_d.rearrange("t p f -> p t f")  # [128, T, FREE] HBM view

            load_ranges = [(0, 1), (1, 2), (2, 4), (4, 8), (8, 12), (12, 16)]
            for a, b in load_ranges:
                t0, f0 = divmod(a * CHF, FREE)
                t1, f1 = divmod(b * CHF, FREE)
                if f0 == 0 and f1 == 0:
                    src = xv[:, t0:t1, :]
                else:
                    assert t1 == t0 and f1 > f0 or (t1 == t0 + 1 and f1 == 0)
                    src = xv[:, t0, f0:f1 if f1 else FREE]
                nc.sync.dma_start(out=xt[:, a * CHF:b * CHF], in_=src)

            w_prev = None
            for t in range(T):
                vs = xt[:, t * FREE:(t + 1) * FREE]  # v_t (in place over X_t)
                o = op.tile([P, FREE], U8, name="o", tag="o")
                wn = (
                    pp.tile([P, FREE], F32, name="w", tag="w")
                    if t < T - 1 else None
                )
                theta = float(2 ** t)

                for i, (lo, hi) in enumerate(spans):
                    # --- update: v_t = w_{t-1} + X_t (in place) ---
                    if t > 0:
                        eng = nc.vector if i == 0 else nc.gpsimd
                        eng.tensor_tensor(
                            vs[:, lo:hi], w_prev[:, lo:hi], vs[:, lo:hi],
                            op=ADD)
                    # --- spikes: o = sign(v - 2^t) as uint8 (byte==1) ---
                    nc.scalar.activation(
                        o[:, lo:hi], vs[:, lo:hi], SIGN,
                        bias=nbias[:, t:t + 1], scale=1.0)
                    # --- store on the scalar HWDGE ring ---
                    if t == T - 1 and i == 0:
                        h = DW // 2
                        nc.scalar.dma_start(out=o_d[t][:, :h], in_=o[:, :h])
                        nc.scalar.dma_start(out=o_d[t][:, h:DW], in_=o[:, h:DW])
                    else:
                        nc.scalar.dma_start(out=o_d[t][:, lo:hi], in_=o[:, lo:hi])

                # --- masks: w_t = (v_t <= 2^t) * v_t, all on DVE; GPSIMD's
                # chunks first so its next-step adds unblock early.
                if wn is not None:
                    for lo, hi in (spans[1], spans[2], spans[0]):
                        nc.vector.scalar_tensor_tensor(
                            wn[:, lo:hi], vs[:, lo:hi], theta, vs[:, lo:hi],
                            op0=ISLE, op1=MULT)
                w_prev = wn

    nc.compile()
    return nc


def _get_nc():
    global _nc_cache
    if _nc_cache is None:
        _nc_cache = _build()
    return _nc_cache


def _run(x: np.ndarray, **spmd_kwargs):
    nc = _get_nc()
    xr = np.asarray(x, dtype=np.float32).reshape(T, BS, C, HW)
    # Pre-scale X_t = 2^t * x_t (exact power-of-two scaling).
    scale = (2.0 ** np.arange(T, dtype=np.float32)).reshape(T, 1, 1, 1)
    xs = xr * scale
    in_maps = [
        {"x": np.ascontiguousarray(xs[:, k * BSH:(k + 1) * BSH]).reshape(T, P, FREE)}
        for k in range(NCORES)
    ]
    res = run_bass_kernel_spmd(nc, in_maps, core_ids=list(range(NCORES)), **spmd_kwargs)
    out = np.empty((T, BS, C, HW), dtype=np.float32)
    for k in range(NCORES):
        ok = res.results[k]["o"].reshape(T, BSH, C, HW)
        out[:, k * BSH:(k + 1) * BSH] = (ok == 1)
    return out.reshape(T * BS, C, 32, 32), res


def kernel(x: np.ndarray) -> np.ndarray:
    out, _ = _run(x)
    return out


# revision 13
# speedup vs baseline: 1.1337x; 1.1337x over previous
"""LIF (leaky integrate-and-fire) spiking recurrence on 8 Trainium2 cores.

Full input x: [T*bs, C, H, W] = [256, 128, 32, 32] f32 with T=8, bs=32.
Recurrence over T only, elementwise elsewhere:
    u_t = TAU * u_{t-1} * (1 - (u_{t-1} > VTH)) + x_t ;  o_t = (u_t > VTH)

Sharding: fully data-parallel over batch (bs=32 -> 4 per core), no
collectives. Each core's [4,128,32,32] per-timestep slab is a flat
[128 partitions, 4096] tile; the whole 16 MiB input stays SBUF-resident
and the state is computed in place over it.

Design notes (measured on HW):
- DMA cap ~426 GB/s/core -> 16.78 MB in + 4.19 MB out ~ 50 us.
- DVE fp32 2-tensor ops run 1x (1.04 ns/elem); the recurrence needs two
  such passes per step (mask w = (v<=2^t)*v fused in one
  scalar_tensor_tensor, update v' = w + X as tensor_tensor), 62 us total.
  That is the kernel's floor: ACT cannot do 2-tensor ops, GPSIMD compute
  degrades concurrent DVE ~2x via SBUF contention (net negative), PE
  fp32 matmul has ~430ns/512col overhead (identity-add too slow), and
  DMA-accumulate costs 2.17x per byte on the SWDGE ring.
- Byte spikes: output is 0/1, so ACT emits s = sign(v - 2^t) as uint8
  (one pass, exact); host maps byte==1 -> 1.0f. 4x less store traffic.
- Scaled state v_t = 2^t*u_t (host pre-scales X_t = 2^t*x_t, exact) makes
  the update a pure add; the whole pipeline is bitwise exact vs f32 ref.
- In-place state over the x slab means steps only chain through the w
  tile: the DVE stream update->mask->update-> never waits on ACT/stores.

Rings: loads on sync HWDGE, spike stores on scalar HWDGE (ACT program
order). ACT signs trail the DVE by one op; stores trail ACT.
"""

import numpy as np

import concourse.tile as tile
from concourse import bacc, mybir
from concourse.bass_utils import run_bass_kernel_spmd

T = 8
BS = 32
C = 128
HW = 32 * 32
NCORES = 8
BSH = BS // NCORES          # 4 batch elements per core
P = 128                     # SBUF partitions
FREE = BSH * C * HW // P    # 4096 f32 per partition per timestep
CHF = 1024                  # load-ramp unit (quarter timestep)
VTH = 1.0
F32 = mybir.dt.float32
U8 = mybir.dt.uint8

_nc_cache = None


def _build():
    nc = bacc.Bacc("TRN2", target_bir_lowering=False, debug=False, num_devices=NCORES)
    x_d = nc.dram_tensor("x", [T, P, FREE], F32, kind="ExternalInput").ap()
    o_d = nc.dram_tensor("o", [T, P, FREE], U8, kind="ExternalOutput").ap()

    ADD = mybir.AluOpType.add
    ISLE = mybir.AluOpType.is_le
    MULT = mybir.AluOpType.mult
    SIGN = mybir.ActivationFunctionType.Sign

    with tile.TileContext(nc) as tc:
        with (
            tc.tile_pool(name="xa", bufs=1) as xa,
            tc.tile_pool(name="pp", bufs=2) as pp,
            tc.tile_pool(name="op", bufs=3) as op,
            tc.tile_pool(name="cp", bufs=1) as cp,
        ):
            nbias = cp.tile([P, T], F32)
            for t in range(T):
                nc.vector.memset(nbias[:, t:t + 1], -float(2 ** t))

            xt = xa.tile([P, T * FREE], F32)
            xv = x_d.rearrange("t p f -> p t f")  # [128, T, FREE] HBM view

            # Ramped loads (units of CHF=1024): tiny first chunk so the
            # t=0 mask starts as early as possible, then big transfers.
            load_ranges = [(0, 1), (1, 2), (2, 4), (4, 8), (8, 16),
                           (16, 24), (24, 32)]
            for a, b in load_ranges:
                t0, f0 = divmod(a * CHF, FREE)
                t1, f1 = divmod(b * CHF, FREE)
                if f0 == 0 and f1 == 0:
                    src = xv[:, t0:t1, :]
                else:
                    assert (t1 == t0 and f1 > f0) or (t1 == t0 + 1 and f1 == 0)
                    src = xv[:, t0, f0:f1 if f1 else FREE]
                nc.sync.dma_start(out=xt[:, a * CHF:b * CHF], in_=src)

            w_prev = None
            for t in range(T):
                vs = xt[:, t * FREE:(t + 1) * FREE]  # v_t (in place over X_t)
                o = op.tile([P, FREE], U8, name="o", tag="o")
                wn = (
                    pp.tile([P, FREE], F32, name="w", tag="w")
                    if t < T - 1 else None
                )
                theta = float(2 ** t)
                # Chunks: t=0 quarters (gated by the load ramp), t=T-1
                # halves (short drain tail), else full-FREE single ops.
                nh = 4 if t == 0 else (2 if t == T - 1 else 1)
                w = FREE // nh
                for c in range(nh):
                    lo, hi = c * w, (c + 1) * w
                    if t > 0:
                        # v_t = w_{t-1} + X_t (in place over the x slab)
                        nc.vector.tensor_tensor(
                            vs[:, lo:hi], w_prev[:, lo:hi], vs[:, lo:hi],
                            op=ADD)
                    if wn is not None:
                        # w_t = (v_t <= 2^t) * v_t
                        nc.vector.scalar_tensor_tensor(
                            wn[:, lo:hi], vs[:, lo:hi], theta, vs[:, lo:hi],
                            op0=ISLE, op1=MULT)
                    # o_t = sign(v_t - 2^t) as uint8; spike iff byte == 1.
                    nc.scalar.activation(
                        o[:, lo:hi], vs[:, lo:hi], SIGN,
                        bias=nbias[:, t:t + 1], scale=1.0)
                    # Store on the scalar HWDGE ring (ACT program order).
                    if t == T - 1:
                        q = w // 2
                        nc.scalar.dma_start(
                            out=o_d[t][:, lo:lo + q], in_=o[:, lo:lo + q])
                        nc.scalar.dma_start(
                            out=o_d[t][:, lo + q:hi], in_=o[:, lo + q:hi])
                    else:
                        nc.scalar.dma_start(
                            out=o_d[t][:, lo:hi], in_=o[:, lo:hi])
                w_prev = wn

    nc.compile()
    return nc


def _get_nc():
    global _nc_cache
    if _nc_cache is None:
        _nc_cache = _build()
    return _nc_cache


def _run(x: np.ndarray, **spmd_kwargs):
    nc = _get_nc()
    xr = np.asarray(x, dtype=np.float32).reshape(T, BS, C, HW)
    # Pre-scale X_t = 2^t * x_t (exact power-of-two scaling).
    scale = (2.0 ** np.arange(T, dtype=np.float32)).reshape(T, 1, 1, 1)
    xs = xr * scale
    in_maps = [
        {"x": np.ascontiguousarray(xs[:, k * BSH:(k + 1) * BSH]).reshape(T, P, FREE)}
        for k in range(NCORES)
    ]
    res = run_bass_kernel_spmd(nc, in_maps, core_ids=list(range(NCORES)), **spmd_kwargs)
    out = np.empty((T, BS, C, HW), dtype=np.float32)
    for k in range(NCORES):
        ok = res.results[k]["o"].reshape(T, BSH, C, HW)
        out[:, k * BSH:(k + 1) * BSH] = (ok == 1)
    return out.reshape(T * BS, C, 32, 32), res


def kernel(x: np.ndarray) -> np.ndarray:
    out, _ = _run(x)
    return out


# revision 14
# speedup vs baseline: 1.1389x; 1.0047x over previous
"""LIF (leaky integrate-and-fire) spiking recurrence on 8 Trainium2 cores.

Full input x: [T*bs, C, H, W] = [256, 128, 32, 32] f32 with T=8, bs=32.
Recurrence over T only, elementwise elsewhere:
    u_t = TAU * u_{t-1} * (1 - (u_{t-1} > VTH)) + x_t ;  o_t = (u_t > VTH)

Sharding: fully data-parallel over batch (bs=32 -> 4 per core), no
collectives. Each core's [4,128,32,32] per-timestep slab is a flat
[128 partitions, 4096] tile; the whole 16 MiB input stays SBUF-resident
and the state is computed in place over it.

Design notes (measured on HW):
- DMA cap ~426 GB/s/core -> 16.78 MB in + 4.19 MB out ~ 50 us.
- DVE fp32 2-tensor ops run 1x (1.04 ns/elem); the recurrence needs two
  such passes per step (mask w = (v<=2^t)*v fused in one
  scalar_tensor_tensor, update v' = w + X as tensor_tensor), 62 us total.
  That is the kernel's floor: ACT cannot do 2-tensor ops, GPSIMD compute
  degrades concurrent DVE ~2x via SBUF contention (net negative), PE
  fp32 matmul has ~430ns/512col overhead (identity-add too slow), and
  DMA-accumulate costs 2.17x per byte on the SWDGE ring.
- Byte spikes: output is 0/1, so ACT emits s = sign(v - 2^t) as uint8
  (one pass, exact); host maps byte==1 -> 1.0f. 4x less store traffic.
- Scaled state v_t = 2^t*u_t (host pre-scales X_t = 2^t*x_t, exact) makes
  the update a pure add; the whole pipeline is bitwise exact vs f32 ref.
- In-place state over the x slab means steps only chain through the w
  tile: the DVE stream update->mask->update-> never waits on ACT/stores.

Rings: loads on sync HWDGE, spike stores on scalar HWDGE (ACT program
order). ACT signs trail the DVE by one op; stores trail ACT.
"""

import numpy as np

import concourse.tile as tile
from concourse import bacc, mybir
from concourse.bass_utils import run_bass_kernel_spmd

T = 8
BS = 32
C = 128
HW = 32 * 32
NCORES = 8
BSH = BS // NCORES          # 4 batch elements per core
P = 128                     # SBUF partitions
FREE = BSH * C * HW // P    # 4096 f32 per partition per timestep
CHF = 1024                  # load-ramp unit (quarter timestep)
VTH = 1.0
F32 = mybir.dt.float32
U8 = mybir.dt.uint8

_nc_cache = None


def _build():
    nc = bacc.Bacc("TRN2", target_bir_lowering=False, debug=False, num_devices=NCORES)
    x_d = nc.dram_tensor("x", [T, P, FREE], F32, kind="ExternalInput").ap()
    o_d = nc.dram_tensor("o", [T, P, FREE], U8, kind="ExternalOutput").ap()

    ADD = mybir.AluOpType.add
    ISLE = mybir.AluOpType.is_le
    MULT = mybir.AluOpType.mult
    SIGN = mybir.ActivationFunctionType.Sign

    with tile.TileContext(nc) as tc:
        with (
            tc.tile_pool(name="xa", bufs=1) as xa,
            tc.tile_pool(name="pp", bufs=2) as pp,
            tc.tile_pool(name="cp", bufs=1) as cp,
        ):
            nbias = cp.tile([P, T], F32)
            for t in range(T):
                nc.vector.memset(nbias[:, t:t + 1], -float(2 ** t))

            xt = xa.tile([P, T * FREE], F32)
            # One flat spike tile for the whole run (32 KiB/partition):
            # dedicated per-step slices mean no pool-rotation semaphores,
            # and stores can drain arbitrarily late.
            ot = xa.tile([P, T * FREE], U8)
            xv = x_d.rearrange("t p f -> p t f")  # [128, T, FREE] HBM view

            # Ramped loads (elements of 4096/slab): fine chunks first so
            # the t=0 mask starts ~9us and steps 1-2 chase the load
            # front; two 4 MB chunks at the end keep semaphore
            # granularity from stalling step 4.
            load_ranges = [(0, 512), (512, 1024), (1024, 2048),
                           (2048, 4096), (4096, 6144), (6144, 8192),
                           (8192, 16384), (16384, 24576), (24576, 32768)]
            for a, b in load_ranges:
                t0, f0 = divmod(a, FREE)
                t1, f1 = divmod(b, FREE)
                if f0 == 0 and f1 == 0:
                    src = xv[:, t0:t1, :]
                else:
                    assert (t1 == t0 and f1 > f0) or (t1 == t0 + 1 and f1 == 0)
                    src = xv[:, t0, f0:f1 if f1 else FREE]
                nc.sync.dma_start(out=xt[:, a:b], in_=src)

            # Per-step compute chunk boundaries: t=0 gated by the fine
            # ramp, steps 1-2 halved to overlap the load front, t=T-1
            # halved for a short drain tail, else full-FREE single ops.
            chunks = {0: (0, 512, 1024, 2048, 4096), 1: (0, 2048, 4096),
                      2: (0, 2048, 4096), T - 1: (0, 2048, 4096)}
            w_prev = None
            for t in range(T):
                vs = xt[:, t * FREE:(t + 1) * FREE]  # v_t (in place over X_t)
                o = ot[:, t * FREE:(t + 1) * FREE]
                wn = (
                    pp.tile([P, FREE], F32, name="w", tag="w")
                    if t < T - 1 else None
                )
                theta = float(2 ** t)
                bounds = chunks.get(t, (0, 4096))
                for lo, hi in zip(bounds[:-1], bounds[1:]):
                    if t > 0:
                        # v_t = w_{t-1} + X_t (in place over the x slab)
                        nc.vector.tensor_tensor(
                            vs[:, lo:hi], w_prev[:, lo:hi], vs[:, lo:hi],
                            op=ADD)
                    if wn is not None:
                        # w_t = (v_t <= 2^t) * v_t
                        nc.vector.scalar_tensor_tensor(
                            wn[:, lo:hi], vs[:, lo:hi], theta, vs[:, lo:hi],
                            op0=ISLE, op1=MULT)
                    # o_t = sign(v_t - 2^t) as uint8; spike iff byte == 1.
                    nc.scalar.activation(
                        o[:, lo:hi], vs[:, lo:hi], SIGN,
                        bias=nbias[:, t:t + 1], scale=1.0)
                    # Store on the sync HWDGE ring: the FIFO runs it after
                    # every load, so stores never steal load bandwidth
                    # while the DVE chain is still being fed.
                    nc.sync.dma_start(out=o_d[t][:, lo:hi], in_=o[:, lo:hi])
                w_prev = wn

    nc.compile()
    return nc


def _get_nc():
    global _nc_cache
    if _nc_cache is None:
        _nc_cache = _build()
    return _nc_cache


def _run(x: np.ndarray, **spmd_kwargs):
    nc = _get_nc()
    xr = np.asarray(x, dtype=np.float32).reshape(T, BS, C, HW)
    # Pre-scale X_t = 2^t * x_t (exact power-of-two scaling).
    scale = (2.0 ** np.arange(T, dtype=np.float32)).reshape(T, 1, 1, 1)
    xs = xr * scale
    in_maps = [
        {"x": np.ascontiguousarray(xs[:, k * BSH:(k + 1) * BSH]).reshape(T, P, FREE)}
        for k in range(NCORES)
    ]
    res = run_bass_kernel_spmd(nc, in_maps, core_ids=list(range(NCORES)), **spmd_kwargs)
    out = np.empty((T, BS, C, HW), dtype=np.float32)
    for k in range(NCORES):
        ok = res.results[k]["o"].reshape(T, BSH, C, HW)
        out[:, k * BSH:(k + 1) * BSH] = (ok == 1)
    return out.reshape(T * BS, C, 32, 32), res


def kernel(x: np.ndarray) -> np.ndarray:
    out, _ = _run(x)
    return out


# revision 16
# speedup vs baseline: 1.2585x; 1.1050x over previous
"""LIF (leaky integrate-and-fire) spiking recurrence on 8 Trainium2 cores.

Full input x: [T*bs, C, H, W] = [256, 128, 32, 32] f32 with T=8, bs=32.
Recurrence over T only, elementwise elsewhere:
    u_t = TAU * u_{t-1} * (1 - (u_{t-1} > VTH)) + x_t ;  o_t = (u_t > VTH)

Sharding: fully data-parallel over batch (bs=32 -> 4 per core), no
collectives. Each core's [4,128,32,32] per-timestep slab is a flat
[128 partitions, 4096] tile; the whole 16 MiB input stays SBUF-resident
and the state is computed in place over it.

Design notes (measured on HW):
- DMA cap ~426 GB/s/core -> 16.78 MB in + 4.19 MB out ~ 50 us.
- DVE fp32 2-tensor ops run 1x (1.04 ns/elem); the recurrence needs two
  such passes per step (mask w = (v<=2^t)*v fused in one
  scalar_tensor_tensor, update v' = w + X as tensor_tensor), 62 us total.
  That is the kernel's floor: ACT cannot do 2-tensor ops, GPSIMD compute
  degrades concurrent DVE ~2x via SBUF contention (net negative), PE
  fp32 matmul has ~430ns/512col overhead (identity-add too slow), and
  DMA-accumulate costs 2.17x per byte on the SWDGE ring.
- Byte spikes: output is 0/1, so ACT emits s = sign(v - 2^t) as uint8
  (one pass, exact); host maps byte==1 -> 1.0f. 4x less store traffic.
- Scaled state v_t = 2^t*u_t (host pre-scales X_t = 2^t*x_t, exact) makes
  the update a pure add; the whole pipeline is bitwise exact vs f32 ref.
- In-place state over the x slab means steps only chain through the w
  tile: the DVE stream update->mask->update-> never waits on ACT/stores.

Rings: loads on sync HWDGE, spike stores on scalar HWDGE (ACT program
order). ACT signs trail the DVE by one op; stores trail ACT.
"""

import numpy as np

import concourse.tile as tile
from concourse import bacc, mybir
from concourse.bass_utils import run_bass_kernel_spmd

T = 8
BS = 32
C = 128
HW = 32 * 32
NCORES = 8
BSH = BS // NCORES          # 4 batch elements per core
P = 128                     # SBUF partitions
FREE = BSH * C * HW // P    # 4096 f32 per partition per timestep
CHF = 1024                  # load-ramp unit (quarter timestep)
VTH = 1.0
F32 = mybir.dt.float32
U8 = mybir.dt.uint8

_nc_cache = None


def _build():
    nc = bacc.Bacc("TRN2", target_bir_lowering=False, debug=False, num_devices=NCORES)
    x_d = nc.dram_tensor("x", [T, P, FREE], F32, kind="ExternalInput").ap()
    o_d = nc.dram_tensor("o", [T, P, FREE], U8, kind="ExternalOutput").ap()

    ADD = mybir.AluOpType.add
    ISLE = mybir.AluOpType.is_le
    MULT = mybir.AluOpType.mult
    SIGN = mybir.ActivationFunctionType.Sign

    with tile.TileContext(nc) as tc:
        with (
            tc.tile_pool(name="xa", bufs=1) as xa,
            tc.tile_pool(name="pp", bufs=2) as pp,
            tc.tile_pool(name="cp", bufs=1) as cp,
        ):
            nbias = cp.tile([P, T], F32)
            for t in range(T):
                nc.vector.memset(nbias[:, t:t + 1], -float(2 ** t))

            xt = xa.tile([P, T * FREE], F32)
            # One flat spike tile for the whole run (32 KiB/partition):
            # dedicated per-step slices mean no pool-rotation semaphores,
            # and stores can drain arbitrarily late.
            ot = xa.tile([P, T * FREE], U8)
            xv = x_d.rearrange("t p f -> p t f")  # [128, T, FREE] HBM view

            # Ramped loads: fine chunks for slab 0 so the t=0 mask starts
            # as soon as possible, then one DMA per slab so each step's
            # update waits on exactly its own slab's semaphore.
            load_ranges = [(0, 512), (512, 1024), (1024, 2048),
                           (2048, 4096), (4096, 6144), (6144, 8192)]
            load_ranges += [(k * FREE, (k + 1) * FREE) for k in range(2, T)]
            for a, b in load_ranges:
                t0, f0 = divmod(a, FREE)
                t1, f1 = divmod(b, FREE)
                if f0 == 0 and f1 == 0:
                    src = xv[:, t0:t1, :]
                else:
                    assert (t1 == t0 and f1 > f0) or (t1 == t0 + 1 and f1 == 0)
                    src = xv[:, t0, f0:f1 if f1 else FREE]
                nc.sync.dma_start(out=xt[:, a:b], in_=src)

            # Per-step compute chunk boundaries: t=0 gated by the fine
            # ramp, steps 1-2 halved to overlap the load front, t=T-1
            # halved for a short drain tail, else full-FREE single ops.
            chunks = {0: (0, 512, 1024, 2048, 4096), 1: (0, 2048, 4096),
                      2: (0, 2048, 4096), T - 1: (0, 1024, 2048, 3072, 4096)}
            w_prev = None
            for t in range(T):
                vs = xt[:, t * FREE:(t + 1) * FREE]  # v_t (in place over X_t)
                o = ot[:, t * FREE:(t + 1) * FREE]
                wn = (
                    pp.tile([P, FREE], F32, name="w", tag="w")
                    if t < T - 1 else None
                )
                theta = float(2 ** t)
                bounds = chunks.get(t, (0, 4096))
                for lo, hi in zip(bounds[:-1], bounds[1:]):
                    if t > 0:
                        # v_t = w_{t-1} + X_t (in place over the x slab)
                        nc.vector.tensor_tensor(
                            vs[:, lo:hi], w_prev[:, lo:hi], vs[:, lo:hi],
                            op=ADD)
                    if wn is not None:
                        # w_t = (v_t <= 2^t) * v_t
                        nc.vector.scalar_tensor_tensor(
                            wn[:, lo:hi], vs[:, lo:hi], theta, vs[:, lo:hi],
                            op0=ISLE, op1=MULT)
                    # o_t = sign(v_t - 2^t) as uint8; spike iff byte == 1.
                    nc.scalar.activation(
                        o[:, lo:hi], vs[:, lo:hi], SIGN,
                        bias=nbias[:, t:t + 1], scale=1.0)
                    # Store on the sync HWDGE ring: the FIFO runs it after
                    # every load, so stores never steal load bandwidth
                    # while the DVE chain is still being fed.
                    nc.sync.dma_start(out=o_d[t][:, lo:hi], in_=o[:, lo:hi])
                w_prev = wn

    nc.compile()
    return nc


def _get_nc():
    global _nc_cache
    if _nc_cache is None:
        _nc_cache = _build()
    return _nc_cache


def _run(x: np.ndarray, **spmd_kwargs):
    nc = _get_nc()
    xr = np.asarray(x, dtype=np.float32).reshape(T, BS, C, HW)
    # Pre-scale X_t = 2^t * x_t (exact power-of-two scaling).
    scale = (2.0 ** np.arange(T, dtype=np.float32)).reshape(T, 1, 1, 1)
    xs = xr * scale
    in_maps = [
        {"x": np.ascontiguousarray(xs[:, k * BSH:(k + 1) * BSH]).reshape(T, P, FREE)}
        for k in range(NCORES)
    ]
    res = run_bass_kernel_spmd(nc, in_maps, core_ids=list(range(NCORES)), **spmd_kwargs)
    out = np.empty((T, BS, C, HW), dtype=np.float32)
    for k in range(NCORES):
        ok = res.results[k]["o"].reshape(T, BSH, C, HW)
        out[:, k * BSH:(k + 1) * BSH] = (ok == 1)
    return out.reshape(T * BS, C, 32, 32), res


def kernel(x: np.ndarray) -> np.ndarray:
    out, _ = _run(x)
    return out


# revision 17
# speedup vs baseline: 1.2610x; 1.0020x over previous
"""LIF (leaky integrate-and-fire) spiking recurrence on 8 Trainium2 cores.

Full input x: [T*bs, C, H, W] = [256, 128, 32, 32] f32 with T=8, bs=32.
Recurrence over T only, elementwise elsewhere:
    u_t = TAU * u_{t-1} * (1 - (u_{t-1} > VTH)) + x_t ;  o_t = (u_t > VTH)

Sharding: fully data-parallel over batch (bs=32 -> 4 per core), no
collectives. Each core's [4,128,32,32] per-timestep slab is a flat
[128 partitions, 4096] tile; the whole 16 MiB input stays SBUF-resident
and the state is computed in place over it.

Design notes (measured on HW):
- DMA cap ~426 GB/s/core -> 16.78 MB in + 4.19 MB out ~ 50 us.
- DVE fp32 2-tensor ops run 1x (1.04 ns/elem); the recurrence needs two
  such passes per step (mask w = (v<=2^t)*v fused in one
  scalar_tensor_tensor, update v' = w + X as tensor_tensor), 62 us total.
  That is the kernel's floor: ACT cannot do 2-tensor ops, GPSIMD compute
  degrades concurrent DVE ~2x via SBUF contention (net negative), PE
  fp32 matmul has ~430ns/512col overhead (identity-add too slow), and
  DMA-accumulate costs 2.17x per byte on the SWDGE ring.
- Byte spikes: output is 0/1, so ACT emits s = sign(v - 2^t) as uint8
  (one pass, exact); host maps byte==1 -> 1.0f. 4x less store traffic.
- Scaled state v_t = 2^t*u_t (host pre-scales X_t = 2^t*x_t, exact) makes
  the update a pure add; the whole pipeline is bitwise exact vs f32 ref.
- In-place state over the x slab means steps only chain through the w
  tile: the DVE stream update->mask->update-> never waits on ACT/stores.

Rings: loads on sync HWDGE, spike stores on scalar HWDGE (ACT program
order). ACT signs trail the DVE by one op; stores trail ACT.
"""

import numpy as np

import concourse.tile as tile
from concourse import bacc, mybir
from concourse.bass_utils import run_bass_kernel_spmd

T = 8
BS = 32
C = 128
HW = 32 * 32
NCORES = 8
BSH = BS // NCORES          # 4 batch elements per core
P = 128                     # SBUF partitions
FREE = BSH * C * HW // P    # 4096 f32 per partition per timestep
CHF = 1024                  # load-ramp unit (quarter timestep)
VTH = 1.0
F32 = mybir.dt.float32
U8 = mybir.dt.uint8

_nc_cache = None


def _build():
    nc = bacc.Bacc("TRN2", target_bir_lowering=False, debug=False, num_devices=NCORES)
    x_d = nc.dram_tensor("x", [T, P, FREE], F32, kind="ExternalInput").ap()
    o_d = nc.dram_tensor("o", [T, P, FREE], U8, kind="ExternalOutput").ap()

    ADD = mybir.AluOpType.add
    ISLE = mybir.AluOpType.is_le
    MULT = mybir.AluOpType.mult
    SIGN = mybir.ActivationFunctionType.Sign

    with tile.TileContext(nc) as tc:
        with (
            tc.tile_pool(name="xa", bufs=1) as xa,
            tc.tile_pool(name="pp", bufs=2) as pp,
            tc.tile_pool(name="cp", bufs=1) as cp,
        ):
            nbias = cp.tile([P, T], F32)
            for t in range(T):
                nc.vector.memset(nbias[:, t:t + 1], -float(2 ** t))

            xt = xa.tile([P, T * FREE], F32)
            # One flat spike tile for the whole run (32 KiB/partition):
            # dedicated per-step slices mean no pool-rotation semaphores,
            # and stores can drain arbitrarily late.
            ot = xa.tile([P, T * FREE], U8)
            xv = x_d.rearrange("t p f -> p t f")  # [128, T, FREE] HBM view

            # Ramped loads: fine chunks for slab 0 so the t=0 mask starts
            # as soon as possible, then one DMA per slab so each step's
            # update waits on exactly its own slab's semaphore.
            load_ranges = [(0, 512), (512, 1024), (1024, 2048),
                           (2048, 4096), (4096, 6144), (6144, 8192)]
            load_ranges += [(k * FREE, (k + 1) * FREE) for k in range(2, T)]
            for a, b in load_ranges:
                t0, f0 = divmod(a, FREE)
                t1, f1 = divmod(b, FREE)
                if f0 == 0 and f1 == 0:
                    src = xv[:, t0:t1, :]
                else:
                    assert (t1 == t0 and f1 > f0) or (t1 == t0 + 1 and f1 == 0)
                    src = xv[:, t0, f0:f1 if f1 else FREE]
                nc.sync.dma_start(out=xt[:, a:b], in_=src)

            # Per-step compute chunk boundaries: t=0 gated by the fine
            # ramp, steps 1-2 halved to overlap the load front, t=T-1
            # halved for a short drain tail, else full-FREE single ops.
            chunks = {0: (0, 512, 1024, 2048, 4096), 1: (0, 2048, 4096),
                      T - 1: (0, 1024, 2048, 3072, 4096)}
            w_prev = None
            for t in range(T):
                vs = xt[:, t * FREE:(t + 1) * FREE]  # v_t (in place over X_t)
                o = ot[:, t * FREE:(t + 1) * FREE]
                wn = (
                    pp.tile([P, FREE], F32, name="w", tag="w")
                    if t < T - 1 else None
                )
                theta = float(2 ** t)
                bounds = chunks.get(t, (0, 4096))
                for lo, hi in zip(bounds[:-1], bounds[1:]):
                    if t > 0:
                        # v_t = w_{t-1} + X_t (in place over the x slab)
                        nc.vector.tensor_tensor(
                            vs[:, lo:hi], w_prev[:, lo:hi], vs[:, lo:hi],
                            op=ADD)
                    if wn is not None:
                        # w_t = (v_t <= 2^t) * v_t
                        nc.vector.scalar_tensor_tensor(
                            wn[:, lo:hi], vs[:, lo:hi], theta, vs[:, lo:hi],
                            op0=ISLE, op1=MULT)
                    # o_t = sign(v_t - 2^t) as uint8; spike iff byte == 1.
                    nc.scalar.activation(
                        o[:, lo:hi], vs[:, lo:hi], SIGN,
                        bias=nbias[:, t:t + 1], scale=1.0)
                    # Store on the sync HWDGE ring: the FIFO runs it after
                    # every load, so stores never steal load bandwidth
                    # while the DVE chain is still being fed.
                    nc.sync.dma_start(out=o_d[t][:, lo:hi], in_=o[:, lo:hi])
                w_prev = wn

    nc.compile()
    return nc


def _get_nc():
    global _nc_cache
    if _nc_cache is None:
        _nc_cache = _build()
    return _nc_cache


def _run(x: np.ndarray, **spmd_kwargs):
    nc = _get_nc()
    xr = np.asarray(x, dtype=np.float32).reshape(T, BS, C, HW)
    # Pre-scale X_t = 2^t * x_t (exact power-of-two scaling).
    scale = (2.0 ** np.arange(T, dtype=np.float32)).reshape(T, 1, 1, 1)
    xs = xr * scale
    in_maps = [
        {"x": np.ascontiguousarray(xs[:, k * BSH:(k + 1) * BSH]).reshape(T, P, FREE)}
        for k in range(NCORES)
    ]
    res = run_bass_kernel_spmd(nc, in_maps, core_ids=list(range(NCORES)), **spmd_kwargs)
    out = np.empty((T, BS, C, HW), dtype=np.float32)
    for k in range(NCORES):
        ok = res.results[k]["o"].reshape(T, BSH, C, HW)
        out[:, k * BSH:(k + 1) * BSH] = (ok == 1)
    return out.reshape(T * BS, C, 32, 32), res


def kernel(x: np.ndarray) -> np.ndarray:
    out, _ = _run(x)
    return out
